# revision 3
# baseline (speedup 1.0000x reference)
"""Multi-head self-attention (B=16,T=512,C=1024,H=16) on 8 NeuronCores.

Strategy: data-parallel over batch (2 batches/core), no collectives.
Schedule keeps the PE dense (HAM stays warm at 2.4GHz) and hides the
scalar-engine exp (the softmax) under the projection matmuls:

  pair p: QK-project head pair p -> scores + exp + mask for 4 (b,h) chains
  V projection + AV + out-projection woven between pairs so every engine
  always has work and nothing big sits at the tail.

Layouts avoid on-device transposes (same tricks as the ancestor kernel):
  - QK projection emits [f, tok]; scores are computed transposed
    sT[kt, qt]; softmax sums come from a ones-column appended to v.
  - normalization uses reciprocal_approx_fast + a DRAM-bounce broadcast,
    fused into the PSUM->SBUF copy of the attention output.
"""

import math
from contextlib import ExitStack

import numpy as np

import concourse.bass as bass
import concourse.mybir as mybir
import concourse.tile as tile
from concourse import bacc
from concourse.bass_utils import run_bass_kernel_spmd

N_CORES = 8
B, T, C = 16, 512, 1024
H = 16
DH = C // H  # 64
B_LOC = B // N_CORES  # 2
TOK = B_LOC * T  # 1024 tokens per core
P = 128
CT = C // P  # 8 contraction tiles
NR = T // P  # 4 kt blocks
DT = mybir.dt.float16
F32 = mybir.dt.float32

# compact pT column offsets per kt-block r (lengths 512,384,256,128)
POFF = [0, 512, 896, 1152]
PTW = 1280


def _build_nc():
    nc = bacc.Bacc("TRN2", target_bir_lowering=False, debug=False,
                   num_devices=N_CORES)

    xT = nc.dram_tensor("xT", [C, TOK], DT, kind="ExternalInput").ap()
    wqk = nc.dram_tensor("wqk", [16, P, CT, P], DT, kind="ExternalInput").ap()
    wv = nc.dram_tensor("wv", [P, 2, CT, 512], DT, kind="ExternalInput").ap()
    wo = nc.dram_tensor("wo", [P, 2, CT, 512], DT, kind="ExternalInput").ap()
    maskd = nc.dram_tensor("maskd", [NR, P, P], DT,
                           kind="ExternalInput").ap()
    kpmb = nc.dram_tensor("kpmb", [B_LOC, T], F32, kind="ExternalInput").ap()
    bias = nc.dram_tensor("bias", [C], F32, kind="ExternalInput").ap()
    out = nc.dram_tensor("out", [TOK, C], F32, kind="ExternalOutput").ap()
    lall = nc.dram_tensor("lall", [B_LOC, H, T], F32).ap()
    linv_scr = nc.dram_tensor("linv_scr", [B_LOC, H, T], DT).ap()

    with tile.TileContext(nc) as tc:
        _emit(nc, tc, xT, wqk, wv, wo, maskd, kpmb, bias, out, lall,
              linv_scr)

    nc.compile()
    return nc


def _emit(nc, tc, xT, wqk, wv, wo, maskd, kpmb, bias, out, lall, linv_scr):
    ctx = ExitStack()
    with ctx:
        singles = ctx.enter_context(tc.tile_pool(name="singles", bufs=1))
        ps_qk = ctx.enter_context(tc.tile_pool(name="ps_qk", bufs=2,
                                               space="PSUM"))
        ps_s = ctx.enter_context(tc.tile_pool(name="ps_s", bufs=1,
                                              space="PSUM"))
        ps_o = ctx.enter_context(tc.tile_pool(name="ps_o", bufs=3,
                                              space="PSUM"))
        wq_pool = ctx.enter_context(tc.tile_pool(name="wq", bufs=3))
        pt_pool = ctx.enter_context(tc.tile_pool(name="pt", bufs=14))
        lrow_pool = ctx.enter_context(tc.tile_pool(name="lrow", bufs=4))
        lp_pool = ctx.enter_context(tc.tile_pool(name="lp", bufs=2))
        lf_pool = ctx.enter_context(tc.tile_pool(name="lf", bufs=4))
        aost_pool = ctx.enter_context(tc.tile_pool(name="aost", bufs=2))
        y_pool = ctx.enter_context(tc.tile_pool(name="y", bufs=3))

        # --- persistent SBUF tensors ---
        qk_sb = singles.tile([P, 16, TOK], DT)             # 32 KB/part
        v_sb = singles.tile([P, TOK // P, H, DH + 1], DT)  # 16.6 KB/part
        ao_b = [singles.tile([P, CT, T], DT, name=f"ao_b{b}")
                for b in range(B_LOC)]                     # 2x 8 KB/part
        wv_sb = singles.tile([P, 2, CT, 512], DT)          # 16 KB/part
        wo_sb = singles.tile([P, 2, CT, 512], DT)          # 16 KB/part
        bias_sb = singles.tile([P, C], F32)                # 4 KB/part
        maskd_sb = singles.tile([P, NR, P], DT)            # 1 KB/part
        kpmb_sb = singles.tile([P, B_LOC * NR], F32)
        xk = [singles.tile([P, TOK], DT, name=f"x_{k}") for k in range(CT)]

        # --- prologue DMAs: just enough for the first QK matmuls first ---
        nc.sync.dma_start(out=xk[0][:], in_=xT[0:P, :])
        wq0 = wq_pool.tile([P, CT, P], DT, tag="wq", name="wq_0")
        nc.sync.dma_start(out=wq0[:], in_=wqk[0])
        for k in range(1, CT):
            nc.sync.dma_start(out=xk[k][:], in_=xT[k * P:(k + 1) * P, :])
        nc.sync.dma_start(out=maskd_sb[:],
                          in_=maskd.rearrange("r p q -> p r q"))
        nc.sync.dma_start(out=kpmb_sb[:],
                          in_=kpmb.rearrange("b (r p) -> p (b r)", p=P))
        bias_bcast = bass.AP(tensor=bias.tensor, offset=bias.offset,
                             ap=[[0, P], *bias.ap])
        nc.gpsimd.dma_start(out=bias_sb[:], in_=bias_bcast)
        nc.vector.memset(v_sb[:, :, :, DH:DH + 1], 1.0)

        def emit_qk(j, wq_tile=None):
            if wq_tile is None:
                wq_tile = wq_pool.tile([P, CT, P], DT, tag="wq",
                                       name=f"wq_{j}")
                nc.sync.dma_start(out=wq_tile[:], in_=wqk[j])
            for tt in range(2):
                ps = ps_qk.tile([P, 512], F32, tag="ps", name=f"ps_qk{j}_{tt}")
                for k in range(CT):
                    nc.tensor.matmul(ps[:], wq_tile[:, k, :],
                                     xk[k][:, tt * 512:(tt + 1) * 512],
                                     start=(k == 0), stop=(k == CT - 1))
                nc.vector.tensor_copy(
                    out=qk_sb[:, j, tt * 512:(tt + 1) * 512], in_=ps[:])

        pt_tiles = {}

        def emit_scores(b, h):
            p = h // 2
            dlo = DH * (h % 2)
            jq, jk = p, 8 + p
            pT = pt_pool.tile([P, PTW], DT, tag="pT", name=f"pT_{b}_{h}")
            pt_tiles[(b, h)] = pT
            sA = ps_s.tile([P, 512], F32, tag="sA", name=f"sA_{b}_{h}")
            sB = ps_s.tile([P, 512], F32, tag="sB", name=f"sB_{b}_{h}")
            sC = ps_s.tile([P, 512], F32, tag="sC", name=f"sC_{b}_{h}")
            sloc = [sA[:, 0:512], sB[:, 0:384], sC[:, 0:256], sB[:, 384:512]]
            for r in range(NR):
                ln = (NR - r) * P
                kT = qk_sb[dlo:dlo + DH, jk,
                           b * T + r * P: b * T + (r + 1) * P]
                qTr = qk_sb[dlo:dlo + DH, jq, b * T + r * P:(b + 1) * T]
                nc.tensor.matmul(sloc[r], kT, qTr, start=True, stop=True)
                nc.scalar.activation(
                    out=pT[:, POFF[r]:POFF[r] + ln], in_=sloc[r],
                    func=mybir.ActivationFunctionType.Exp,
                    bias=kpmb_sb[:, b * NR + r: b * NR + r + 1])
                nc.vector.tensor_mul(
                    out=pT[:, POFF[r]:POFF[r] + P],
                    in0=pT[:, POFF[r]:POFF[r] + P],
                    in1=maskd_sb[:, r, :])

        def emit_v(n):
            for m in range(TOK // P):
                ps = ps_qk.tile([P, 512], F32, tag="ps", name=f"ps_v{n}_{m}")
                for k in range(CT):
                    nc.tensor.matmul(
                        ps[:], xk[k][:, m * P:(m + 1) * P], wv_sb[:, n, k, :],
                        start=(k == 0), stop=(k == CT - 1))
                nc.vector.tensor_copy(
                    out=v_sb[:, m, 8 * n:8 * n + 8, 0:DH],
                    in_=ps[:].rearrange("p (h d) -> p h d", d=DH))

        def emit_av_pair(b, p):
            pos = []
            for kap in range(2):
                h = 2 * p + kap
                pT = pt_tiles.pop((b, h))
                po = ps_o.tile([P, 512], F32, tag="po", name=f"po_{b}_{h}")
                for r in range(NR):
                    ln = (NR - r) * P
                    nc.tensor.matmul(po[0:DH + 1, r * P:],
                                     v_sb[:, b * NR + r, h, :],
                                     pT[:, POFF[r]:POFF[r] + ln],
                                     start=(r == 0), stop=(r == NR - 1))
                lrow = lrow_pool.tile([DH + 1, 512], F32, tag="lrow",
                                      name=f"lrow_{b}_{h}")
                nc.scalar.copy(out=lrow[DH:DH + 1, :],
                               in_=po[DH:DH + 1, :])
                nc.gpsimd.dma_start(out=lall[b, h, :],
                                    in_=lrow[DH:DH + 1, :])
                pos.append(po)
            # 1/l for both heads, broadcast via DRAM bounce
            lpair = lp_pool.tile([2, 512], F32, tag="lpair",
                                 name=f"lp_{b}_{p}")
            nc.gpsimd.dma_start(out=lpair[:], in_=lall[b, 2 * p:2 * p + 2, :])
            lpinv = lp_pool.tile([2, 512], F32, tag="lpinv",
                                 name=f"lpi_{b}_{p}")
            nc.vector.reciprocal_approx_fast(out=lpinv[:], in_=lpair[:])
            lpd = lp_pool.tile([2, 512], DT, tag="lpd", name=f"lpd_{b}_{p}")
            nc.vector.tensor_copy(out=lpd[:], in_=lpinv[:])
            nc.gpsimd.dma_start(out=linv_scr[b, 2 * p:2 * p + 2, :],
                                in_=lpd[:])
            for kap in range(2):
                h = 2 * p + kap
                lf = lf_pool.tile([DH, 512], DT, tag="lf",
                                  name=f"lf_{b}_{h}")
                src = bass.AP(tensor=linv_scr.tensor,
                              offset=linv_scr.offset + (b * H + h) * T,
                              ap=[[0, DH], [1, T]])
                nc.gpsimd.dma_start(out=lf[:], in_=src)
                if kap == 0:
                    nc.vector.tensor_mul(out=ao_b[b][0:DH, p, :],
                                         in0=pos[kap][0:DH, :], in1=lf[:])
                else:
                    ao_st = aost_pool.tile([DH, 512], DT, tag="aost",
                                           name=f"aost_{b}_{p}")
                    nc.vector.tensor_mul(out=ao_st[:], in0=pos[kap][0:DH, :],
                                         in1=lf[:])
                    nc.sync.dma_start(out=ao_b[b][DH:P, p, :], in_=ao_st[:])

        def yproj_chunk(b, i):
            n, m = i // NR, i % NR
            ps = ps_qk.tile([P, 512], F32, tag="ps", name=f"ps_y{b}_{n}_{m}")
            for k in range(CT):
                nc.tensor.matmul(ps[:], ao_b[b][:, k, m * P:(m + 1) * P],
                                 wo_sb[:, n, k, :],
                                 start=(k == 0), stop=(k == CT - 1))
            y = y_pool.tile([P, 512], F32, tag="y")
            nc.vector.tensor_add(out=y[:], in0=ps[:],
                                 in1=bias_sb[:, n * 512:(n + 1) * 512])
            nc.sync.dma_start(
                out=out[b * T + m * P: b * T + (m + 1) * P,
                        n * 512:(n + 1) * 512],
                in_=y[:])

        # --- main interleaved schedule ---
        for p in range(8):
            emit_qk(p, wq0 if p == 0 else None)
            emit_qk(8 + p)
            for b in range(B_LOC):
                for kap in range(2):
                    emit_scores(b, 2 * p + kap)
            if p == 0:
                nc.sync.dma_start(out=wv_sb[:, 0], in_=wv[:, 0])
            elif p == 1:
                emit_v(0)
            elif p == 2:
                nc.sync.dma_start(out=wv_sb[:, 1], in_=wv[:, 1])
                for b in range(B_LOC):
                    emit_av_pair(b, 0)
            elif p == 3:
                for b in range(B_LOC):
                    emit_av_pair(b, 1)
            elif p == 4:
                emit_v(1)
                nc.sync.dma_start(out=wo_sb[:, 0], in_=wo[:, 0])
                nc.sync.dma_start(out=wo_sb[:, 1], in_=wo[:, 1])
            elif p == 5:
                for b in range(B_LOC):
                    emit_av_pair(b, 2)
            elif p == 6:
                for b in range(B_LOC):
                    emit_av_pair(b, 3)
        for q in range(4, 8):
            for b in range(B_LOC):
                emit_av_pair(b, q)
        for b in range(B_LOC):
            for i in range(2 * NR):
                yproj_chunk(b, i)


_NC_CACHE = None


def _get_nc():
    global _NC_CACHE
    if _NC_CACHE is None:
        _NC_CACHE = _build_nc()
    return _NC_CACHE


def _prep_core_inputs(x, mask, key_padding_mask, w_qkv, w_out, b_out):
    """Host-side sharding + layout prep. Returns list of per-core in_maps."""
    x = np.asarray(x, dtype=np.float32)
    mask = np.asarray(mask)
    kpm = np.asarray(key_padding_mask)
    w_qkv = np.asarray(w_qkv, dtype=np.float32)
    w_out = np.asarray(w_out, dtype=np.float32)
    b_out = np.asarray(b_out, dtype=np.float32)

    scale = 1.0 / math.sqrt(DH)
    w2 = w_qkv[:2 * C].copy()
    w2[:C] *= scale  # fold 1/sqrt(dh) into the Q weights
    # [j, p, k, f]: contiguous 2KB/partition DMA per j-tile
    wqk_r = np.ascontiguousarray(
        w2.reshape(16, P, CT, P).transpose(0, 3, 2, 1).astype(np.float16))
    # wv/wo as [p, n, k, f]: contiguous per-partition lines
    wv_r = np.ascontiguousarray(
        w_qkv[2 * C:].T.reshape(CT, P, 2, 512).transpose(1, 2, 0, 3)
        .astype(np.float16))
    wo_r = np.ascontiguousarray(
        w_out.T.reshape(CT, P, 2, 512).transpose(1, 2, 0, 3)
        .astype(np.float16))

    # The kernel exploits the causal structure: it only applies mask values
    # inside the diagonal 128x128 blocks and zero-fills fully-masked blocks.
    exp_tril = np.tril(np.ones((T, T), dtype=mask.dtype))
    assert np.array_equal(mask, exp_tril), "kernel assumes causal tril mask"
    maskTf = mask.T.astype(np.float16)  # [kt, qt]
    maskd = np.stack([maskTf[r * P:(r + 1) * P, r * P:(r + 1) * P]
                      for r in range(NR)])  # [NR, P, P]

    in_maps = []
    for i in range(N_CORES):
        xs = x[i * B_LOC:(i + 1) * B_LOC]      # [B_LOC, T, C]
        xT = np.ascontiguousarray(xs.reshape(TOK, C).T.astype(np.float16))
        kb = np.where(kpm[i * B_LOC:(i + 1) * B_LOC], -1e30,
                      0.0).astype(np.float32)  # [B_LOC, T]
        in_maps.append({
            "xT": xT,
            "wqk": wqk_r,
            "wv": wv_r,
            "wo": wo_r,
            "maskd": np.ascontiguousarray(maskd),
            "kpmb": kb,
            "bias": b_out,
        })
    return in_maps


def kernel(x, mask, key_padding_mask, w_qkv, w_out, b_out, _trace=False,
           _tmpdir=None):
    nc = _get_nc()
    in_maps = _prep_core_inputs(x, mask, key_padding_mask, w_qkv, w_out, b_out)
    res = run_bass_kernel_spmd(nc, in_maps, list(range(N_CORES)),
                               trace=_trace, tmpdir=_tmpdir)
    outs = [res.results[i]["out"].reshape(B_LOC, T, C) for i in range(N_CORES)]
    full = np.concatenate(outs, axis=0).astype(np.float32)
    kernel._last_exec_time_ns = res.exec_time_ns
    return full


# revision 23
# speedup vs baseline: 1.0301x; 1.0301x over previous
"""Multi-head self-attention (B=16,T=512,C=1024,H=16) on 8 NeuronCores.

Strategy: data-parallel over batch (2 batches/core), no collectives.
Schedule keeps the PE dense (HAM stays warm at 2.4GHz) and hides the
scalar-engine exp (the softmax) under the projection matmuls:

  pair p: QK-project head pair p -> scores + exp + mask for 4 (b,h) chains
  V projection + AV + out-projection woven between pairs so every engine
  always has work and nothing big sits at the tail.

Layouts avoid on-device transposes (same tricks as the ancestor kernel):
  - QK projection emits [f, tok]; scores are computed transposed
    sT[kt, qt]; softmax sums come from a ones-column appended to v.
  - 1/l via reciprocal_approx_fast straight out of PSUM, broadcast across
    partitions with a stride-0 DRAM-bounce DMA; the in-place normalize of
    the attention output and the out-projection bias add run on the
    otherwise-idle gpsimd engine.
Engine roles: PE matmuls; scalar exp + PSUM->SBUF drains of the attention
output; vector QK/V drains, diag masks, reciprocals; gpsimd normalize +
bias; sync queue small/latency DMAs; scalar queue bulk weight DMAs.
"""

import math
from contextlib import ExitStack

import numpy as np

import concourse.bass as bass
import concourse.mybir as mybir
import concourse.tile as tile
from concourse import bacc
from concourse.bass_utils import run_bass_kernel_spmd

_DEBUG = False

N_CORES = 8
B, T, C = 16, 512, 1024
H = 16
DH = C // H  # 64
B_LOC = B // N_CORES  # 2
TOK = B_LOC * T  # 1024 tokens per core
P = 128
CT = C // P  # 8 contraction tiles
NR = T // P  # 4 kt blocks
DT = mybir.dt.float16
F32 = mybir.dt.float32

# compact pT column offsets per kt-block r (lengths 512,384,256,128)
POFF = [0, 512, 896, 1152]
PTW = 1280


def _build_nc():
    nc = bacc.Bacc("TRN2", target_bir_lowering=False, debug=False,
                   num_devices=N_CORES)

    xT = nc.dram_tensor("xT", [C, TOK], DT, kind="ExternalInput").ap()
    wqk = nc.dram_tensor("wqk", [16, P, CT, P], DT, kind="ExternalInput").ap()
    wv = nc.dram_tensor("wv", [P, 2, CT, 512], DT, kind="ExternalInput").ap()
    wo = nc.dram_tensor("wo", [P, 2, CT, 512], DT, kind="ExternalInput").ap()
    maskd = nc.dram_tensor("maskd", [NR, P, P], DT,
                           kind="ExternalInput").ap()
    kpmb = nc.dram_tensor("kpmb", [B_LOC, T], F32, kind="ExternalInput").ap()
    bias = nc.dram_tensor("bias", [C], F32, kind="ExternalInput").ap()
    out = nc.dram_tensor("out", [TOK, C], F32, kind="ExternalOutput").ap()
    if _DEBUG:
        linv = nc.dram_tensor("linv", [B_LOC, H, T], DT,
                              kind="ExternalOutput").ap()
    else:
        linv = nc.dram_tensor("linv", [B_LOC, H, T], DT).ap()
    dbg = None
    if _DEBUG:
        dbg = {
            "dbg_qk": nc.dram_tensor("dbg_qk", [P, 16, TOK], DT,
                                     kind="ExternalOutput").ap(),
            "dbg_v": nc.dram_tensor("dbg_v", [P, TOK // P, H, DH + 1], DT,
                                    kind="ExternalOutput").ap(),
            "dbg_ao": nc.dram_tensor("dbg_ao", [B_LOC, P, CT, T], DT,
                                     kind="ExternalOutput").ap(),
        }

    with tile.TileContext(nc) as tc:
        _emit(nc, tc, xT, wqk, wv, wo, maskd, kpmb, bias, out, linv, dbg)

    nc.compile()
    return nc


def _emit(nc, tc, xT, wqk, wv, wo, maskd, kpmb, bias, out, linv, dbg=None):
    ctx = ExitStack()
    with ctx:
        singles = ctx.enter_context(tc.tile_pool(name="singles", bufs=1))
        ps_qk = ctx.enter_context(tc.tile_pool(name="ps_qk", bufs=2,
                                               space="PSUM"))
        ps_s = ctx.enter_context(tc.tile_pool(name="ps_s", bufs=1,
                                              space="PSUM"))
        ps_o = ctx.enter_context(tc.tile_pool(name="ps_o", bufs=2,
                                              space="PSUM"))
        wq_pool = ctx.enter_context(tc.tile_pool(name="wq", bufs=3))
        pt_pool = ctx.enter_context(tc.tile_pool(name="pt", bufs=14))
        lrow_pool = ctx.enter_context(tc.tile_pool(name="lrow", bufs=4))
        lf_pool = ctx.enter_context(tc.tile_pool(name="lf", bufs=3))
        aost_pool = ctx.enter_context(tc.tile_pool(name="aost", bufs=3))
        y_pool = ctx.enter_context(tc.tile_pool(name="y", bufs=3))

        # --- persistent SBUF tensors ---
        qk_sb = singles.tile([P, 16, TOK], DT)             # 32 KB/part
        v_sb = singles.tile([P, TOK // P, H, DH + 1], DT)  # 16.6 KB/part
        ao_b = [singles.tile([P, CT, T], DT, name=f"ao_b{b}")
                for b in range(B_LOC)]                     # 2x 8 KB/part
        wv_sb = [[singles.tile([P, 512], DT, name=f"wv_{n}_{k}")
                  for k in range(CT)] for n in range(2)]   # 16 KB/part
        wo_sb = [[singles.tile([P, 512], DT, name=f"wo_{n}_{k}")
                  for k in range(CT)] for n in range(2)]   # 16 KB/part
        bias_sb = singles.tile([P, C], F32)                # 4 KB/part
        maskd_sb = singles.tile([P, NR, P], DT)            # 1 KB/part
        kpmb_sb = singles.tile([P, B_LOC * NR], F32)
        xk = [singles.tile([P, TOK], DT, name=f"x_{k}") for k in range(CT)]

        # --- prologue DMAs: x split over both queues, first wq first ---
        nc.sync.dma_start(out=xk[0][:], in_=xT[0:P, :])
        wq0 = wq_pool.tile([P, CT, P], DT, tag="wq", name="wq_0")
        nc.sync.dma_start(out=wq0[:], in_=wqk[0])
        for k in range(1, CT):
            nc.sync.dma_start(out=xk[k][:], in_=xT[k * P:(k + 1) * P, :])
        nc.sync.dma_start(out=maskd_sb[:],
                          in_=maskd.rearrange("r p q -> p r q"))
        nc.sync.dma_start(out=kpmb_sb[:],
                          in_=kpmb.rearrange("b (r p) -> p (b r)", p=P))
        bias_bcast = bass.AP(tensor=bias.tensor, offset=bias.offset,
                             ap=[[0, P], *bias.ap])
        nc.gpsimd.dma_start(out=bias_sb[:], in_=bias_bcast)
        nc.vector.memset(v_sb[:, :, :, DH:DH + 1], 1.0)

        def emit_qk(j, wq_tile=None):
            if wq_tile is None:
                wq_tile = wq_pool.tile([P, CT, P], DT, tag="wq",
                                       name=f"wq_{j}")
                nc.sync.dma_start(out=wq_tile[:], in_=wqk[j])
            for tt in range(2):
                ps = ps_qk.tile([P, 512], F32, tag="ps", name=f"ps_qk{j}_{tt}")
                for k in range(CT):
                    nc.tensor.matmul(ps[:], wq_tile[:, k, :],
                                     xk[k][:, tt * 512:(tt + 1) * 512],
                                     start=(k == 0), stop=(k == CT - 1))
                nc.vector.tensor_copy(
                    out=qk_sb[:, j, tt * 512:(tt + 1) * 512], in_=ps[:])

        pt_tiles = {}

        def emit_scores(b, h):
            p = h // 2
            dlo = DH * (h % 2)
            jq, jk = p, 8 + p
            pT = pt_pool.tile([P, PTW], DT, tag="pT", name=f"pT_{b}_{h}")
            pt_tiles[(b, h)] = pT
            sloc = [ps_s.tile([P, 512], F32, tag=f"s{r}",
                              name=f"s{r}_{b}_{h}")[:, 0:(NR - r) * P]
                    for r in range(NR)]
            for r in range(NR):
                ln = (NR - r) * P
                kT = qk_sb[dlo:dlo + DH, jk,
                           b * T + r * P: b * T + (r + 1) * P]
                qTr = qk_sb[dlo:dlo + DH, jq, b * T + r * P:(b + 1) * T]
                nc.tensor.matmul(sloc[r], kT, qTr, start=True, stop=True)
                nc.scalar.activation(
                    out=pT[:, POFF[r]:POFF[r] + ln], in_=sloc[r],
                    func=mybir.ActivationFunctionType.Exp,
                    bias=kpmb_sb[:, b * NR + r: b * NR + r + 1])
                nc.vector.tensor_mul(
                    out=pT[:, POFF[r]:POFF[r] + P],
                    in0=pT[:, POFF[r]:POFF[r] + P],
                    in1=maskd_sb[:, r, :])

        def emit_v(n):
            for m in range(TOK // P):
                ps = ps_qk.tile([P, 512], F32, tag="ps", name=f"ps_v{n}_{m}")
                for k in range(CT):
                    nc.tensor.matmul(
                        ps[:], xk[k][:, m * P:(m + 1) * P], wv_sb[n][k][:],
                        start=(k == 0), stop=(k == CT - 1))
                nc.vector.tensor_copy(
                    out=v_sb[:, m, 8 * n:8 * n + 8, 0:DH],
                    in_=ps[:].rearrange("p (h d) -> p h d", d=DH))

        def emit_av_pair(b, p):
            # one tile holds 1/l for both heads on their data partitions
            lf = lf_pool.tile([P, 512], DT, tag="lf", name=f"lf_{b}_{p}")
            for kap in range(2):
                h = 2 * p + kap
                pT = pt_tiles.pop((b, h))
                po = ps_o.tile([P, 512], F32, tag="po", name=f"po_{b}_{h}")
                for r in range(NR):
                    ln = (NR - r) * P
                    nc.tensor.matmul(po[0:DH + 1, r * P:],
                                     v_sb[:, b * NR + r, h, :],
                                     pT[:, POFF[r]:POFF[r] + ln],
                                     start=(r == 0), stop=(r == NR - 1))
                # drain PSUM fast: 1/l on vector, data cast on scalar
                lrow = lrow_pool.tile([DH + 1, 512], F32, tag="lrow",
                                      name=f"lrow_{b}_{h}")
                nc.vector.tensor_copy(out=lrow[DH:DH + 1, :],
                                      in_=po[DH:DH + 1, :])
                lrowi = lrow_pool.tile([DH + 1, 512], F32, tag="lrowi",
                                       name=f"lrowi_{b}_{h}")
                nc.vector.reciprocal(out=lrowi[DH:DH + 1, :],
                                     in_=lrow[DH:DH + 1, :])
                lrowd = lrow_pool.tile([DH + 1, 512], DT, tag="lrowd",
                                       name=f"lrowd_{b}_{h}")
                nc.vector.tensor_copy(out=lrowd[DH:DH + 1, :],
                                      in_=lrowi[DH:DH + 1, :])
                nc.sync.dma_start(out=linv[b, h, :], in_=lrowd[DH:DH + 1, :])
                src = bass.AP(tensor=linv.tensor,
                              offset=linv.offset + (b * H + h) * T,
                              ap=[[0, DH], [1, T]])
                nc.sync.dma_start(out=lf[kap * DH:(kap + 1) * DH, :], in_=src)
                if kap == 0:
                    nc.vector.tensor_copy(out=ao_b[b][0:DH, p, :],
                                          in_=po[0:DH, :])
                else:
                    ao_st = aost_pool.tile([DH, 512], DT, tag="aost",
                                           name=f"aost_{b}_{p}")
                    nc.vector.tensor_copy(out=ao_st[:], in_=po[0:DH, :])
                    nc.sync.dma_start(out=ao_b[b][DH:P, p, :], in_=ao_st[:])
            # in-place normalize of both heads on gpsimd (off critical path)
            nc.vector.tensor_mul(out=ao_b[b][:, p, :], in0=ao_b[b][:, p, :],
                                 in1=lf[:])

        def yproj_chunk(b, i):
            n, m = i // NR, i % NR
            ps = ps_qk.tile([P, 512], F32, tag="ps", name=f"ps_y{b}_{n}_{m}")
            for k in range(CT):
                nc.tensor.matmul(ps[:], ao_b[b][:, k, m * P:(m + 1) * P],
                                 wo_sb[n][k][:],
                                 start=(k == 0), stop=(k == CT - 1))
            y = y_pool.tile([P, 512], F32, tag="y")
            nc.vector.tensor_add(out=y[:], in0=ps[:],
                                 in1=bias_sb[:, n * 512:(n + 1) * 512])
            nc.sync.dma_start(
                out=out[b * T + m * P: b * T + (m + 1) * P,
                        n * 512:(n + 1) * 512],
                in_=y[:])

        # --- main interleaved schedule ---
        for p in range(8):
            emit_qk(p, wq0 if p == 0 else None)
            emit_qk(8 + p)
            for b in range(B_LOC):
                for kap in range(2):
                    emit_scores(b, 2 * p + kap)
            if p == 0:
                for k in range(CT):
                    nc.sync.dma_start(out=wv_sb[0][k][:], in_=wv[:, 0, k])
            elif p == 1:
                emit_v(0)
            elif p == 2:
                for k in range(CT):
                    nc.sync.dma_start(out=wv_sb[1][k][:], in_=wv[:, 1, k])
                for b in range(B_LOC):
                    emit_av_pair(b, 0)
            elif p == 3:
                for b in range(B_LOC):
                    emit_av_pair(b, 1)
            elif p == 4:
                emit_v(1)
                for n in range(2):
                    for k in range(CT):
                        nc.sync.dma_start(out=wo_sb[n][k][:],
                                          in_=wo[:, n, k])
            elif p == 5:
                for b in range(B_LOC):
                    emit_av_pair(b, 2)
            elif p == 6:
                for b in range(B_LOC):
                    emit_av_pair(b, 3)
        for q in range(4, 8):
            for b in range(B_LOC):
                emit_av_pair(b, q)
        for b in range(B_LOC):
            for i in range(2 * NR):
                yproj_chunk(b, i)
        if dbg is not None:
            nc.sync.dma_start(out=dbg["dbg_qk"][:], in_=qk_sb[:])

            nc.sync.dma_start(out=dbg["dbg_v"][:], in_=v_sb[:])
            for b in range(B_LOC):
                nc.sync.dma_start(out=dbg["dbg_ao"][b], in_=ao_b[b][:])


_NC_CACHE = None


def _get_nc():
    global _NC_CACHE
    if _NC_CACHE is None:
        _NC_CACHE = _build_nc()
    return _NC_CACHE


def _prep_core_inputs(x, mask, key_padding_mask, w_qkv, w_out, b_out):
    """Host-side sharding + layout prep. Returns list of per-core in_maps."""
    x = np.asarray(x, dtype=np.float32)
    mask = np.asarray(mask)
    kpm = np.asarray(key_padding_mask)
    w_qkv = np.asarray(w_qkv, dtype=np.float32)
    w_out = np.asarray(w_out, dtype=np.float32)
    b_out = np.asarray(b_out, dtype=np.float32)

    scale = 1.0 / math.sqrt(DH)
    w2 = w_qkv[:2 * C].copy()
    w2[:C] *= scale  # fold 1/sqrt(dh) into the Q weights
    # [j, p, k, f]: contiguous 2KB/partition DMA per j-tile
    wqk_r = np.ascontiguousarray(
        w2.reshape(16, P, CT, P).transpose(0, 3, 2, 1).astype(np.float16))
    # wv/wo as [p, n, k, f]: contiguous per-partition lines
    wv_r = np.ascontiguousarray(
        w_qkv[2 * C:].T.reshape(CT, P, 2, 512).transpose(1, 2, 0, 3)
        .astype(np.float16))
    wo_r = np.ascontiguousarray(
        w_out.T.reshape(CT, P, 2, 512).transpose(1, 2, 0, 3)
        .astype(np.float16))

    # The kernel exploits the causal structure: it only applies mask values
    # inside the diagonal 128x128 blocks and zero-fills fully-masked blocks.
    exp_tril = np.tril(np.ones((T, T), dtype=mask.dtype))
    assert np.array_equal(mask, exp_tril), "kernel assumes causal tril mask"
    maskTf = mask.T.astype(np.float16)  # [kt, qt]
    maskd = np.stack([maskTf[r * P:(r + 1) * P, r * P:(r + 1) * P]
                      for r in range(NR)])  # [NR, P, P]

    in_maps = []
    for i in range(N_CORES):
        xs = x[i * B_LOC:(i + 1) * B_LOC]      # [B_LOC, T, C]
        xT = np.ascontiguousarray(xs.reshape(TOK, C).T.astype(np.float16))
        kb = np.where(kpm[i * B_LOC:(i + 1) * B_LOC], -1e30,
                      0.0).astype(np.float32)  # [B_LOC, T]
        in_maps.append({
            "xT": xT,
            "wqk": wqk_r,
            "wv": wv_r,
            "wo": wo_r,
            "maskd": np.ascontiguousarray(maskd),
            "kpmb": kb,
            "bias": b_out,
        })
    return in_maps


def kernel(x, mask, key_padding_mask, w_qkv, w_out, b_out, _trace=False,
           _tmpdir=None):
    nc = _get_nc()
    in_maps = _prep_core_inputs(x, mask, key_padding_mask, w_qkv, w_out, b_out)
    res = run_bass_kernel_spmd(nc, in_maps, list(range(N_CORES)),
                               trace=_trace, tmpdir=_tmpdir)
    outs = [res.results[i]["out"].reshape(B_LOC, T, C) for i in range(N_CORES)]
    full = np.concatenate(outs, axis=0).astype(np.float32)
    kernel._last_exec_time_ns = res.exec_time_ns
    return full


# revision 30
# speedup vs baseline: 1.4320x; 1.3902x over previous
"""Multi-head self-attention (B=16,T=512,C=1024,H=16) on 8 NeuronCores.

Strategy: data-parallel over batch (2 batches/core), no collectives.
Schedule keeps the PE dense (HAM stays warm at 2.4GHz) and hides the
scalar-engine exp (the softmax) under the projection matmuls:

  pair p: QK-project head pair p -> scores + exp + mask for 4 (b,h) chains
  V projection + AV + out-projection woven between pairs so every engine
  always has work and nothing big sits at the tail.

Layouts avoid on-device transposes (same tricks as the ancestor kernel):
  - QK projection emits [f, tok]; scores are computed transposed
    sT[kt, qt]; softmax sums come from a ones-column appended to v.
  - 1/l via reciprocal_approx_fast straight out of PSUM, broadcast across
    partitions with a stride-0 DRAM-bounce DMA; the in-place normalize of
    the attention output and the out-projection bias add run on the
    otherwise-idle gpsimd engine.
Engine roles: PE matmuls; scalar exp + PSUM->SBUF drains of the attention
output; vector QK/V drains, diag masks, reciprocals; gpsimd normalize +
bias; sync queue small/latency DMAs; scalar queue bulk weight DMAs.
"""

import math
from contextlib import ExitStack

import numpy as np

import concourse.bass as bass
import concourse.mybir as mybir
import concourse.tile as tile
from concourse import bacc
from concourse.bass_utils import run_bass_kernel_spmd

_DEBUG = False

N_CORES = 8
B, T, C = 16, 512, 1024
H = 16
DH = C // H  # 64
B_LOC = B // N_CORES  # 2
TOK = B_LOC * T  # 1024 tokens per core
P = 128
CT = C // P  # 8 contraction tiles
NR = T // P  # 4 kt blocks
DT = mybir.dt.float16
F32 = mybir.dt.float32

# compact pT column offsets per kt-block r (lengths 512,384,256,128)
POFF = [0, 512, 896, 1152]
PTW = 1280


def _build_nc():
    nc = bacc.Bacc("TRN2", target_bir_lowering=False, debug=False,
                   num_devices=N_CORES)

    xT = nc.dram_tensor("xT", [C, TOK], DT, kind="ExternalInput").ap()
    wqk = nc.dram_tensor("wqk", [16, P, CT, P], DT, kind="ExternalInput").ap()
    wv = nc.dram_tensor("wv", [P, 2, CT, 512], DT, kind="ExternalInput").ap()
    wo = nc.dram_tensor("wo", [P, 2, CT, 512], DT, kind="ExternalInput").ap()
    maskd = nc.dram_tensor("maskd", [NR, P, P], DT,
                           kind="ExternalInput").ap()
    kpmb = nc.dram_tensor("kpmb", [B_LOC, T], F32, kind="ExternalInput").ap()
    bias = nc.dram_tensor("bias", [C], F32, kind="ExternalInput").ap()
    out = nc.dram_tensor("out", [TOK, C], F32, kind="ExternalOutput").ap()
    lall = nc.dram_tensor("lall", [B_LOC, H, T], F32).ap()
    if _DEBUG:
        linv = nc.dram_tensor("linv", [B_LOC, H, T], DT,
                              kind="ExternalOutput").ap()
    else:
        linv = nc.dram_tensor("linv", [B_LOC, H, T], DT).ap()
    dbg = None
    if _DEBUG:
        dbg = {
            "dbg_qk": nc.dram_tensor("dbg_qk", [P, 16, TOK], DT,
                                     kind="ExternalOutput").ap(),
            "dbg_v": nc.dram_tensor("dbg_v", [P, TOK // P, H, DH + 1], DT,
                                    kind="ExternalOutput").ap(),
            "dbg_ao": nc.dram_tensor("dbg_ao", [B_LOC, P, CT, T], DT,
                                     kind="ExternalOutput").ap(),
        }

    with tile.TileContext(nc) as tc:
        _emit(nc, tc, xT, wqk, wv, wo, maskd, kpmb, bias, out, lall, linv,
              dbg)

    nc.compile()
    return nc


def _emit(nc, tc, xT, wqk, wv, wo, maskd, kpmb, bias, out, lall, linv,
          dbg=None):
    ctx = ExitStack()
    with ctx:
        singles = ctx.enter_context(tc.tile_pool(name="singles", bufs=1))
        ps_qk = ctx.enter_context(tc.tile_pool(name="ps_qk", bufs=2,
                                               space="PSUM"))
        ps_s = ctx.enter_context(tc.tile_pool(name="ps_s", bufs=1,
                                              space="PSUM"))
        ps_o = ctx.enter_context(tc.tile_pool(name="ps_o", bufs=2,
                                              space="PSUM"))
        wq_pool = ctx.enter_context(tc.tile_pool(name="wq", bufs=3))
        pt_pool = ctx.enter_context(tc.tile_pool(name="pt", bufs=14))
        lrow_pool = ctx.enter_context(tc.tile_pool(name="lrow", bufs=4))
        lf_pool = ctx.enter_context(tc.tile_pool(name="lf", bufs=3))
        aost_pool = ctx.enter_context(tc.tile_pool(name="aost", bufs=3))
        y_pool = ctx.enter_context(tc.tile_pool(name="y", bufs=3))

        # --- persistent SBUF tensors ---
        qk_sb = singles.tile([P, 16, TOK], DT)             # 32 KB/part
        v_sb = singles.tile([P, TOK // P, H, DH + 1], DT)  # 16.6 KB/part
        ao_b = [singles.tile([P, CT, T], DT, name=f"ao_b{b}")
                for b in range(B_LOC)]                     # 2x 8 KB/part
        wv_sb = [[singles.tile([P, 512], DT, name=f"wv_{n}_{k}")
                  for k in range(CT)] for n in range(2)]   # 16 KB/part
        wo_sb = [[singles.tile([P, 512], DT, name=f"wo_{n}_{k}")
                  for k in range(CT)] for n in range(2)]   # 16 KB/part
        bias_sb = singles.tile([P, C], F32)                # 4 KB/part
        maskd_sb = singles.tile([P, NR, P], DT)            # 1 KB/part
        kpmb_sb = singles.tile([P, B_LOC * NR], F32)
        xk = [singles.tile([P, TOK], DT, name=f"x_{k}") for k in range(CT)]

        # --- prologue DMAs: x split over both queues, first wq first ---
        nc.sync.dma_start(out=xk[0][:], in_=xT[0:P, :])
        wq0 = wq_pool.tile([P, CT, P], DT, tag="wq", name="wq_0")
        nc.sync.dma_start(out=wq0[:], in_=wqk[0])
        for k in range(1, CT):
            nc.sync.dma_start(out=xk[k][:], in_=xT[k * P:(k + 1) * P, :])
        nc.sync.dma_start(out=maskd_sb[:],
                          in_=maskd.rearrange("r p q -> p r q"))
        nc.sync.dma_start(out=kpmb_sb[:],
                          in_=kpmb.rearrange("b (r p) -> p (b r)", p=P))
        bias_bcast = bass.AP(tensor=bias.tensor, offset=bias.offset,
                             ap=[[0, P], *bias.ap])
        nc.gpsimd.dma_start(out=bias_sb[:], in_=bias_bcast)
        nc.vector.memset(v_sb[:, :, :, DH:DH + 1], 1.0)

        def emit_qk(j, wq_tile=None):
            if wq_tile is None:
                wq_tile = wq_pool.tile([P, CT, P], DT, tag="wq",
                                       name=f"wq_{j}")
                nc.sync.dma_start(out=wq_tile[:], in_=wqk[j])
            for tt in range(2):
                ps = ps_qk.tile([P, 512], F32, tag="ps", name=f"ps_qk{j}_{tt}")
                for k in range(CT):
                    nc.tensor.matmul(ps[:], wq_tile[:, k, :],
                                     xk[k][:, tt * 512:(tt + 1) * 512],
                                     start=(k == 0), stop=(k == CT - 1))
                nc.vector.tensor_copy(
                    out=qk_sb[:, j, tt * 512:(tt + 1) * 512], in_=ps[:])

        pt_tiles = {}

        def emit_scores(b, h):
            p = h // 2
            dlo = DH * (h % 2)
            jq, jk = p, 8 + p
            pT = pt_pool.tile([P, PTW], DT, tag="pT", name=f"pT_{b}_{h}")
            pt_tiles[(b, h)] = pT
            sloc = [ps_s.tile([P, 512], F32, tag=f"s{r}",
                              name=f"s{r}_{b}_{h}")[:, 0:(NR - r) * P]
                    for r in range(NR)]
            for r in range(NR):
                ln = (NR - r) * P
                kT = qk_sb[dlo:dlo + DH, jk,
                           b * T + r * P: b * T + (r + 1) * P]
                qTr = qk_sb[dlo:dlo + DH, jq, b * T + r * P:(b + 1) * T]
                nc.tensor.matmul(sloc[r], kT, qTr, start=True, stop=True)
                nc.scalar.activation(
                    out=pT[:, POFF[r]:POFF[r] + ln], in_=sloc[r],
                    func=mybir.ActivationFunctionType.Exp,
                    bias=kpmb_sb[:, b * NR + r: b * NR + r + 1])
                nc.vector.tensor_mul(
                    out=pT[:, POFF[r]:POFF[r] + P],
                    in0=pT[:, POFF[r]:POFF[r] + P],
                    in1=maskd_sb[:, r, :])

        def emit_v(n):
            for m in range(TOK // P):
                ps = ps_qk.tile([P, 512], F32, tag="ps", name=f"ps_v{n}_{m}")
                for k in range(CT):
                    nc.tensor.matmul(
                        ps[:], xk[k][:, m * P:(m + 1) * P], wv_sb[n][k][:],
                        start=(k == 0), stop=(k == CT - 1))
                nc.vector.tensor_copy(
                    out=v_sb[:, m, 8 * n:8 * n + 8, 0:DH],
                    in_=ps[:].rearrange("p (h d) -> p h d", d=DH))

        def emit_av_pair(b, p):
            for kap in range(2):
                h = 2 * p + kap
                pT = pt_tiles.pop((b, h))
                po = ps_o.tile([P, 512], F32, tag="po", name=f"po_{b}_{h}")
                for r in range(NR):
                    ln = (NR - r) * P
                    nc.tensor.matmul(po[0:DH + 1, r * P:],
                                     v_sb[:, b * NR + r, h, :],
                                     pT[:, POFF[r]:POFF[r] + ln],
                                     start=(r == 0), stop=(r == NR - 1))
                # drain PSUM fast; raw row sums parked in DRAM, 1/l batched
                # later in emit_norm_half off the critical path
                lrow = lrow_pool.tile([DH + 1, 512], F32, tag="lrow",
                                      name=f"lrow_{b}_{h}")
                nc.vector.tensor_copy(out=lrow[DH:DH + 1, :],
                                      in_=po[DH:DH + 1, :])
                nc.sync.dma_start(out=lall[b, h, :], in_=lrow[DH:DH + 1, :])
                if kap == 0:
                    nc.vector.tensor_copy(out=ao_b[b][0:DH, p, :],
                                          in_=po[0:DH, :])
                else:
                    ao_st = aost_pool.tile([DH, 512], DT, tag="aost",
                                           name=f"aost_{b}_{p}")
                    nc.vector.tensor_copy(out=ao_st[:], in_=po[0:DH, :])
                    nc.sync.dma_start(out=ao_b[b][DH:P, p, :], in_=ao_st[:])

        def emit_norm_half(b, half):
            """1/l for 8 heads at once, then normalize their 4 ao c-tiles."""
            hs = slice(half * 8, half * 8 + 8)
            lpart = lrow_pool.tile([8, T], F32, tag="lpart",
                                   name=f"lpart_{b}_{half}")
            nc.sync.dma_start(out=lpart[:], in_=lall[b, hs])
            nc.vector.reciprocal(out=lpart[:], in_=lpart[:])
            lpartd = lrow_pool.tile([8, T], DT, tag="lpartd",
                                    name=f"lpartd_{b}_{half}")
            nc.vector.tensor_copy(out=lpartd[:], in_=lpart[:])
            nc.sync.dma_start(out=linv[b, hs], in_=lpartd[:])
            for p in range(half * 4, half * 4 + 4):
                lf = lf_pool.tile([P, 512], DT, tag="lf", name=f"lf_{b}_{p}")
                for kap in range(2):
                    h = 2 * p + kap
                    src = bass.AP(tensor=linv.tensor,
                                  offset=linv.offset + (b * H + h) * T,
                                  ap=[[0, DH], [1, T]])
                    nc.sync.dma_start(out=lf[kap * DH:(kap + 1) * DH, :],
                                      in_=src)
                nc.vector.tensor_mul(out=ao_b[b][:, p, :],
                                     in0=ao_b[b][:, p, :], in1=lf[:])

        def yproj_chunk(b, i):
            n, m = i // NR, i % NR
            ps = ps_qk.tile([P, 512], F32, tag="ps", name=f"ps_y{b}_{n}_{m}")
            for k in range(CT):
                nc.tensor.matmul(ps[:], ao_b[b][:, k, m * P:(m + 1) * P],
                                 wo_sb[n][k][:],
                                 start=(k == 0), stop=(k == CT - 1))
            y = y_pool.tile([P, 512], F32, tag="y")
            nc.vector.tensor_add(out=y[:], in0=ps[:],
                                 in1=bias_sb[:, n * 512:(n + 1) * 512])
            nc.sync.dma_start(
                out=out[b * T + m * P: b * T + (m + 1) * P,
                        n * 512:(n + 1) * 512],
                in_=y[:])

        # --- main interleaved schedule ---
        for p in range(8):
            emit_qk(p, wq0 if p == 0 else None)
            emit_qk(8 + p)
            for b in range(B_LOC):
                for kap in range(2):
                    emit_scores(b, 2 * p + kap)
            if p == 0:
                for k in range(CT):
                    nc.sync.dma_start(out=wv_sb[0][k][:], in_=wv[:, 0, k])
            elif p == 1:
                emit_v(0)
            elif p == 2:
                for k in range(CT):
                    nc.sync.dma_start(out=wv_sb[1][k][:], in_=wv[:, 1, k])
                for b in range(B_LOC):
                    emit_av_pair(b, 0)
            elif p == 3:
                for b in range(B_LOC):
                    emit_av_pair(b, 1)
            elif p == 4:
                emit_v(1)
                for n in range(2):
                    for k in range(CT):
                        nc.sync.dma_start(out=wo_sb[n][k][:],
                                          in_=wo[:, n, k])
            elif p == 5:
                for b in range(B_LOC):
                    emit_av_pair(b, 2)
            elif p == 6:
                for b in range(B_LOC):
                    emit_av_pair(b, 3)
            elif p == 7:
                for b in range(B_LOC):
                    emit_norm_half(b, 0)
        for q in range(4, 8):
            for b in range(B_LOC):
                emit_av_pair(b, q)
        for b in range(B_LOC):
            emit_norm_half(b, 1)
        for b in range(B_LOC):
            for i in range(2 * NR):
                yproj_chunk(b, i)
        if dbg is not None:
            nc.sync.dma_start(out=dbg["dbg_qk"][:], in_=qk_sb[:])

            nc.sync.dma_start(out=dbg["dbg_v"][:], in_=v_sb[:])
            for b in range(B_LOC):
                nc.sync.dma_start(out=dbg["dbg_ao"][b], in_=ao_b[b][:])


_NC_CACHE = None


def _get_nc():
    global _NC_CACHE
    if _NC_CACHE is None:
        _NC_CACHE = _build_nc()
    return _NC_CACHE


def _prep_core_inputs(x, mask, key_padding_mask, w_qkv, w_out, b_out):
    """Host-side sharding + layout prep. Returns list of per-core in_maps."""
    x = np.asarray(x, dtype=np.float32)
    mask = np.asarray(mask)
    kpm = np.asarray(key_padding_mask)
    w_qkv = np.asarray(w_qkv, dtype=np.float32)
    w_out = np.asarray(w_out, dtype=np.float32)
    b_out = np.asarray(b_out, dtype=np.float32)

    scale = 1.0 / math.sqrt(DH)
    w2 = w_qkv[:2 * C].copy()
    w2[:C] *= scale  # fold 1/sqrt(dh) into the Q weights
    # [j, p, k, f]: contiguous 2KB/partition DMA per j-tile
    wqk_r = np.ascontiguousarray(
        w2.reshape(16, P, CT, P).transpose(0, 3, 2, 1).astype(np.float16))
    # wv/wo as [p, n, k, f]: contiguous per-partition lines
    wv_r = np.ascontiguousarray(
        w_qkv[2 * C:].T.reshape(CT, P, 2, 512).transpose(1, 2, 0, 3)
        .astype(np.float16))
    wo_r = np.ascontiguousarray(
        w_out.T.reshape(CT, P, 2, 512).transpose(1, 2, 0, 3)
        .astype(np.float16))

    # The kernel exploits the causal structure: it only applies mask values
    # inside the diagonal 128x128 blocks and zero-fills fully-masked blocks.
    exp_tril = np.tril(np.ones((T, T), dtype=mask.dtype))
    assert np.array_equal(mask, exp_tril), "kernel assumes causal tril mask"
    maskTf = mask.T.astype(np.float16)  # [kt, qt]
    maskd = np.stack([maskTf[r * P:(r + 1) * P, r * P:(r + 1) * P]
                      for r in range(NR)])  # [NR, P, P]

    in_maps = []
    for i in range(N_CORES):
        xs = x[i * B_LOC:(i + 1) * B_LOC]      # [B_LOC, T, C]
        xT = np.ascontiguousarray(xs.reshape(TOK, C).T.astype(np.float16))
        kb = np.where(kpm[i * B_LOC:(i + 1) * B_LOC], -1e30,
                      0.0).astype(np.float32)  # [B_LOC, T]
        in_maps.append({
            "xT": xT,
            "wqk": wqk_r,
            "wv": wv_r,
            "wo": wo_r,
            "maskd": np.ascontiguousarray(maskd),
            "kpmb": kb,
            "bias": b_out,
        })
    return in_maps


def kernel(x, mask, key_padding_mask, w_qkv, w_out, b_out, _trace=False,
           _tmpdir=None):
    nc = _get_nc()
    in_maps = _prep_core_inputs(x, mask, key_padding_mask, w_qkv, w_out, b_out)
    res = run_bass_kernel_spmd(nc, in_maps, list(range(N_CORES)),
                               trace=_trace, tmpdir=_tmpdir)
    outs = [res.results[i]["out"].reshape(B_LOC, T, C) for i in range(N_CORES)]
    full = np.concatenate(outs, axis=0).astype(np.float32)
    kernel._last_exec_time_ns = res.exec_time_ns
    return full


# revision 31
# speedup vs baseline: 1.5016x; 1.0486x over previous
"""Multi-head self-attention (B=16,T=512,C=1024,H=16) on 8 NeuronCores.

Strategy: data-parallel over batch (2 batches/core), no collectives.
Schedule keeps the PE dense (HAM stays warm at 2.4GHz) and hides the
scalar-engine exp (the softmax) under the projection matmuls:

  pair p: QK-project head pair p -> scores + exp + mask for 4 (b,h) chains
  V projection + AV + out-projection woven between pairs so every engine
  always has work and nothing big sits at the tail.

Layouts avoid on-device transposes (same tricks as the ancestor kernel):
  - QK projection emits [f, tok]; scores are computed transposed
    sT[kt, qt]; softmax sums come from a ones-column appended to v.
  - 1/l via reciprocal_approx_fast straight out of PSUM, broadcast across
    partitions with a stride-0 DRAM-bounce DMA; the in-place normalize of
    the attention output and the out-projection bias add run on the
    otherwise-idle gpsimd engine.
Engine roles: PE matmuls; scalar exp + PSUM->SBUF drains of the attention
output; vector QK/V drains, diag masks, reciprocals; gpsimd normalize +
bias; sync queue small/latency DMAs; scalar queue bulk weight DMAs.
"""

import math
from contextlib import ExitStack

import numpy as np

import concourse.bass as bass
import concourse.mybir as mybir
import concourse.tile as tile
from concourse import bacc
from concourse.bass_utils import run_bass_kernel_spmd

_DEBUG = False

N_CORES = 8
B, T, C = 16, 512, 1024
H = 16
DH = C // H  # 64
B_LOC = B // N_CORES  # 2
TOK = B_LOC * T  # 1024 tokens per core
P = 128
CT = C // P  # 8 contraction tiles
NR = T // P  # 4 kt blocks
DT = mybir.dt.float16
F32 = mybir.dt.float32

# compact pT column offsets per kt-block r (lengths 512,384,256,128)
POFF = [0, 512, 896, 1152]
PTW = 1280


def _build_nc():
    nc = bacc.Bacc("TRN2", target_bir_lowering=False, debug=False,
                   num_devices=N_CORES)

    xT = nc.dram_tensor("xT", [C, TOK], DT, kind="ExternalInput").ap()
    wqk = nc.dram_tensor("wqk", [16, P, CT, P], DT, kind="ExternalInput").ap()
    wv = nc.dram_tensor("wv", [P, 2, CT, 512], DT, kind="ExternalInput").ap()
    wo = nc.dram_tensor("wo", [P, 2, CT, 512], DT, kind="ExternalInput").ap()
    maskd = nc.dram_tensor("maskd", [NR, P, P], DT,
                           kind="ExternalInput").ap()
    kpmb = nc.dram_tensor("kpmb", [B_LOC, T], F32, kind="ExternalInput").ap()
    bias = nc.dram_tensor("bias", [C], F32, kind="ExternalInput").ap()
    out = nc.dram_tensor("out", [TOK, C], F32, kind="ExternalOutput").ap()
    lall = nc.dram_tensor("lall", [B_LOC, H, T], F32).ap()
    if _DEBUG:
        linv = nc.dram_tensor("linv", [B_LOC, H, T], DT,
                              kind="ExternalOutput").ap()
    else:
        linv = nc.dram_tensor("linv", [B_LOC, H, T], DT).ap()
    dbg = None
    if _DEBUG:
        dbg = {
            "dbg_qk": nc.dram_tensor("dbg_qk", [P, 16, TOK], DT,
                                     kind="ExternalOutput").ap(),
            "dbg_v": nc.dram_tensor("dbg_v", [P, TOK // P, H, DH + 1], DT,
                                    kind="ExternalOutput").ap(),
            "dbg_ao": nc.dram_tensor("dbg_ao", [B_LOC, P, CT, T], DT,
                                     kind="ExternalOutput").ap(),
        }

    with tile.TileContext(nc) as tc:
        _emit(nc, tc, xT, wqk, wv, wo, maskd, kpmb, bias, out, lall, linv,
              dbg)

    nc.compile()
    return nc


def _emit(nc, tc, xT, wqk, wv, wo, maskd, kpmb, bias, out, lall, linv,
          dbg=None):
    ctx = ExitStack()
    with ctx:
        singles = ctx.enter_context(tc.tile_pool(name="singles", bufs=1))
        ps_qk = ctx.enter_context(tc.tile_pool(name="ps_qk", bufs=2,
                                               space="PSUM"))
        ps_s = ctx.enter_context(tc.tile_pool(name="ps_s", bufs=1,
                                              space="PSUM"))
        ps_o = ctx.enter_context(tc.tile_pool(name="ps_o", bufs=3,
                                              space="PSUM"))
        wq_pool = ctx.enter_context(tc.tile_pool(name="wq", bufs=3))
        pt_pool = ctx.enter_context(tc.tile_pool(name="pt", bufs=14))
        lrow_pool = ctx.enter_context(tc.tile_pool(name="lrow", bufs=4))
        lf_pool = ctx.enter_context(tc.tile_pool(name="lf", bufs=3))
        aost_pool = ctx.enter_context(tc.tile_pool(name="aost", bufs=3))
        y_pool = ctx.enter_context(tc.tile_pool(name="y", bufs=3))

        # --- persistent SBUF tensors ---
        qk_sb = singles.tile([P, 16, TOK], DT)             # 32 KB/part
        v_sb = singles.tile([P, TOK // P, H, DH + 1], DT)  # 16.6 KB/part
        ao_b = [singles.tile([P, CT, T], DT, name=f"ao_b{b}")
                for b in range(B_LOC)]                     # 2x 8 KB/part
        wv_sb = [[singles.tile([P, 512], DT, name=f"wv_{n}_{k}")
                  for k in range(CT)] for n in range(2)]   # 16 KB/part
        wo_sb = [[singles.tile([P, 512], DT, name=f"wo_{n}_{k}")
                  for k in range(CT)] for n in range(2)]   # 16 KB/part
        bias_sb = singles.tile([P, C], F32)                # 4 KB/part
        maskd_sb = singles.tile([P, NR, P], DT)            # 1 KB/part
        kpmb_sb = singles.tile([P, B_LOC * NR], F32)
        xk = [singles.tile([P, TOK], DT, name=f"x_{k}") for k in range(CT)]

        # --- prologue DMAs: x split over both queues, first wq first ---
        nc.sync.dma_start(out=xk[0][:], in_=xT[0:P, :])
        wq0 = wq_pool.tile([P, CT, P], DT, tag="wq", name="wq_0")
        nc.sync.dma_start(out=wq0[:], in_=wqk[0])
        for k in range(1, CT):
            eng = nc.sync if k % 2 == 0 else nc.scalar
            eng.dma_start(out=xk[k][:], in_=xT[k * P:(k + 1) * P, :])
        nc.scalar.dma_start(out=maskd_sb[:],
                            in_=maskd.rearrange("r p q -> p r q"))
        nc.scalar.dma_start(out=kpmb_sb[:],
                            in_=kpmb.rearrange("b (r p) -> p (b r)", p=P))
        bias_bcast = bass.AP(tensor=bias.tensor, offset=bias.offset,
                             ap=[[0, P], *bias.ap])
        nc.gpsimd.dma_start(out=bias_sb[:], in_=bias_bcast)
        nc.vector.memset(v_sb[:, :, :, DH:DH + 1], 1.0)

        def emit_qk(j, wq_tile=None):
            if wq_tile is None:
                wq_tile = wq_pool.tile([P, CT, P], DT, tag="wq",
                                       name=f"wq_{j}")
                nc.sync.dma_start(out=wq_tile[:], in_=wqk[j])
            for tt in range(2):
                ps = ps_qk.tile([P, 512], F32, tag="ps", name=f"ps_qk{j}_{tt}")
                for k in range(CT):
                    nc.tensor.matmul(ps[:], wq_tile[:, k, :],
                                     xk[k][:, tt * 512:(tt + 1) * 512],
                                     start=(k == 0), stop=(k == CT - 1))
                nc.vector.tensor_copy(
                    out=qk_sb[:, j, tt * 512:(tt + 1) * 512], in_=ps[:])

        pt_tiles = {}

        def emit_scores(b, h):
            p = h // 2
            dlo = DH * (h % 2)
            jq, jk = p, 8 + p
            pT = pt_pool.tile([P, PTW], DT, tag="pT", name=f"pT_{b}_{h}")
            pt_tiles[(b, h)] = pT
            sA = ps_s.tile([P, 512], F32, tag="sA", name=f"sA_{b}_{h}")
            sB = ps_s.tile([P, 512], F32, tag="sB", name=f"sB_{b}_{h}")
            sC = ps_s.tile([P, 512], F32, tag="sC", name=f"sC_{b}_{h}")
            sloc = [sA[:, 0:512], sB[:, 0:384], sC[:, 0:256], sB[:, 384:512]]
            for r in range(NR):
                ln = (NR - r) * P
                kT = qk_sb[dlo:dlo + DH, jk,
                           b * T + r * P: b * T + (r + 1) * P]
                qTr = qk_sb[dlo:dlo + DH, jq, b * T + r * P:(b + 1) * T]
                # r==3 shares sB's bank with r==1: start=False so it does not
                # re-zero the 2KB zero-region holding r1's live data; its own
                # bytes are still pending-zero from r1's start.
                nc.tensor.matmul(sloc[r], kT, qTr, start=(r != 3), stop=True,
                                 skip_group_check=(r == 3))
                nc.scalar.activation(
                    out=pT[:, POFF[r]:POFF[r] + ln], in_=sloc[r],
                    func=mybir.ActivationFunctionType.Exp,
                    bias=kpmb_sb[:, b * NR + r: b * NR + r + 1])
                nc.vector.tensor_mul(
                    out=pT[:, POFF[r]:POFF[r] + P],
                    in0=pT[:, POFF[r]:POFF[r] + P],
                    in1=maskd_sb[:, r, :])

        def emit_v(n):
            for m in range(TOK // P):
                ps = ps_qk.tile([P, 512], F32, tag="ps", name=f"ps_v{n}_{m}")
                for k in range(CT):
                    nc.tensor.matmul(
                        ps[:], xk[k][:, m * P:(m + 1) * P], wv_sb[n][k][:],
                        start=(k == 0), stop=(k == CT - 1))
                nc.vector.tensor_copy(
                    out=v_sb[:, m, 8 * n:8 * n + 8, 0:DH],
                    in_=ps[:].rearrange("p (h d) -> p h d", d=DH))

        def emit_av_pair(b, p):
            for kap in range(2):
                h = 2 * p + kap
                pT = pt_tiles.pop((b, h))
                po = ps_o.tile([P, 512], F32, tag="po", name=f"po_{b}_{h}")
                for r in range(NR):
                    ln = (NR - r) * P
                    nc.tensor.matmul(po[0:DH + 1, r * P:],
                                     v_sb[:, b * NR + r, h, :],
                                     pT[:, POFF[r]:POFF[r] + ln],
                                     start=(r == 0), stop=(r == NR - 1))
                # drain PSUM fast; raw row sums parked in DRAM, 1/l batched
                # later in emit_norm_half off the critical path
                lrow = lrow_pool.tile([DH + 1, 512], F32, tag="lrow",
                                      name=f"lrow_{b}_{h}")
                nc.vector.tensor_copy(out=lrow[DH:DH + 1, :],
                                      in_=po[DH:DH + 1, :])
                nc.sync.dma_start(out=lall[b, h, :], in_=lrow[DH:DH + 1, :])
                if kap == 0:
                    nc.vector.tensor_copy(out=ao_b[b][0:DH, p, :],
                                          in_=po[0:DH, :])
                else:
                    ao_st = aost_pool.tile([DH, 512], DT, tag="aost",
                                           name=f"aost_{b}_{p}")
                    nc.vector.tensor_copy(out=ao_st[:], in_=po[0:DH, :])
                    nc.sync.dma_start(out=ao_b[b][DH:P, p, :], in_=ao_st[:])

        def emit_norm_half(b, half):
            """1/l for 8 heads at once, then normalize their 4 ao c-tiles."""
            hs = slice(half * 8, half * 8 + 8)
            lpart = lrow_pool.tile([8, T], F32, tag="lpart",
                                   name=f"lpart_{b}_{half}")
            nc.sync.dma_start(out=lpart[:], in_=lall[b, hs])
            nc.vector.reciprocal(out=lpart[:], in_=lpart[:])
            lpartd = lrow_pool.tile([8, T], DT, tag="lpartd",
                                    name=f"lpartd_{b}_{half}")
            nc.vector.tensor_copy(out=lpartd[:], in_=lpart[:])
            nc.sync.dma_start(out=linv[b, hs], in_=lpartd[:])
            lf4 = lf_pool.tile([P, 4, 512], DT, tag="lf4",
                               name=f"lf4_{b}_{half}")
            base = linv.offset + (b * H + half * 8) * T
            for kap in range(2):
                src_ap = bass.AP(tensor=linv.tensor, offset=base + kap * T,
                                 ap=[[0, DH], [2 * T, 4], [1, T]])
                nc.sync.dma_start(out=lf4[kap * DH:(kap + 1) * DH, :, :],
                                  in_=src_ap)
            for pp in range(4):
                p = half * 4 + pp
                nc.vector.tensor_mul(out=ao_b[b][:, p, :],
                                     in0=ao_b[b][:, p, :],
                                     in1=lf4[:, pp, :])

        def yproj_chunk(b, i):
            n, m = i // NR, i % NR
            ps = ps_qk.tile([P, 512], F32, tag="ps", name=f"ps_y{b}_{n}_{m}")
            for k in range(CT):
                nc.tensor.matmul(ps[:], ao_b[b][:, k, m * P:(m + 1) * P],
                                 wo_sb[n][k][:],
                                 start=(k == 0), stop=(k == CT - 1))
            y = y_pool.tile([P, 512], F32, tag="y")
            nc.vector.tensor_add(out=y[:], in0=ps[:],
                                 in1=bias_sb[:, n * 512:(n + 1) * 512])
            nc.scalar.dma_start(
                out=out[b * T + m * P: b * T + (m + 1) * P,
                        n * 512:(n + 1) * 512],
                in_=y[:])

        # --- main interleaved schedule ---
        for p in range(8):
            emit_qk(p, wq0 if p == 0 else None)
            emit_qk(8 + p)
            for b in range(B_LOC):
                for kap in range(2):
                    emit_scores(b, 2 * p + kap)
            if p == 0:
                for k in range(CT):
                    nc.sync.dma_start(out=wv_sb[0][k][:], in_=wv[:, 0, k])
            elif p == 1:
                emit_v(0)
            elif p == 2:
                for k in range(CT):
                    nc.sync.dma_start(out=wv_sb[1][k][:], in_=wv[:, 1, k])
                for b in range(B_LOC):
                    emit_av_pair(b, 0)
            elif p == 3:
                for b in range(B_LOC):
                    emit_av_pair(b, 1)
            elif p == 4:
                emit_v(1)
                for n in range(2):
                    for k in range(CT):
                        nc.sync.dma_start(out=wo_sb[n][k][:],
                                          in_=wo[:, n, k])
            elif p == 5:
                for b in range(B_LOC):
                    emit_av_pair(b, 2)
            elif p == 6:
                for b in range(B_LOC):
                    emit_av_pair(b, 3)
            elif p == 7:
                for b in range(B_LOC):
                    emit_norm_half(b, 0)
        for q in range(4, 8):
            emit_av_pair(0, q)
        emit_norm_half(0, 1)
        for q in range(4, 8):
            emit_av_pair(1, q)
        emit_norm_half(1, 1)
        for b in range(B_LOC):
            for i in range(2 * NR):
                yproj_chunk(b, i)
        if dbg is not None:
            nc.sync.dma_start(out=dbg["dbg_qk"][:], in_=qk_sb[:])

            nc.sync.dma_start(out=dbg["dbg_v"][:], in_=v_sb[:])
            for b in range(B_LOC):
                nc.sync.dma_start(out=dbg["dbg_ao"][b], in_=ao_b[b][:])


_NC_CACHE = None


def _get_nc():
    global _NC_CACHE
    if _NC_CACHE is None:
        _NC_CACHE = _build_nc()
    return _NC_CACHE


def _prep_core_inputs(x, mask, key_padding_mask, w_qkv, w_out, b_out):
    """Host-side sharding + layout prep. Returns list of per-core in_maps."""
    x = np.asarray(x, dtype=np.float32)
    mask = np.asarray(mask)
    kpm = np.asarray(key_padding_mask)
    w_qkv = np.asarray(w_qkv, dtype=np.float32)
    w_out = np.asarray(w_out, dtype=np.float32)
    b_out = np.asarray(b_out, dtype=np.float32)

    scale = 1.0 / math.sqrt(DH)
    w2 = w_qkv[:2 * C].copy()
    w2[:C] *= scale  # fold 1/sqrt(dh) into the Q weights
    # [j, p, k, f]: contiguous 2KB/partition DMA per j-tile
    wqk_r = np.ascontiguousarray(
        w2.reshape(16, P, CT, P).transpose(0, 3, 2, 1).astype(np.float16))
    # wv/wo as [p, n, k, f]: contiguous per-partition lines
    wv_r = np.ascontiguousarray(
        w_qkv[2 * C:].T.reshape(CT, P, 2, 512).transpose(1, 2, 0, 3)
        .astype(np.float16))
    wo_r = np.ascontiguousarray(
        w_out.T.reshape(CT, P, 2, 512).transpose(1, 2, 0, 3)
        .astype(np.float16))

    # The kernel exploits the causal structure: it only applies mask values
    # inside the diagonal 128x128 blocks and zero-fills fully-masked blocks.
    exp_tril = np.tril(np.ones((T, T), dtype=mask.dtype))
    assert np.array_equal(mask, exp_tril), "kernel assumes causal tril mask"
    maskTf = mask.T.astype(np.float16)  # [kt, qt]
    maskd = np.stack([maskTf[r * P:(r + 1) * P, r * P:(r + 1) * P]
                      for r in range(NR)])  # [NR, P, P]

    in_maps = []
    for i in range(N_CORES):
        xs = x[i * B_LOC:(i + 1) * B_LOC]      # [B_LOC, T, C]
        xT = np.ascontiguousarray(xs.reshape(TOK, C).T.astype(np.float16))
        kb = np.where(kpm[i * B_LOC:(i + 1) * B_LOC], -1e30,
                      0.0).astype(np.float32)  # [B_LOC, T]
        in_maps.append({
            "xT": xT,
            "wqk": wqk_r,
            "wv": wv_r,
            "wo": wo_r,
            "maskd": np.ascontiguousarray(maskd),
            "kpmb": kb,
            "bias": b_out,
        })
    return in_maps


def kernel(x, mask, key_padding_mask, w_qkv, w_out, b_out, _trace=False,
           _tmpdir=None):
    nc = _get_nc()
    in_maps = _prep_core_inputs(x, mask, key_padding_mask, w_qkv, w_out, b_out)
    res = run_bass_kernel_spmd(nc, in_maps, list(range(N_CORES)),
                               trace=_trace, tmpdir=_tmpdir)
    outs = [res.results[i]["out"].reshape(B_LOC, T, C) for i in range(N_CORES)]
    full = np.concatenate(outs, axis=0).astype(np.float32)
    kernel._last_exec_time_ns = res.exec_time_ns
    return full
